# revision 1
# baseline (speedup 1.0000x reference)
"""3-layer GCN on 8 Trainium2 NeuronCores (Bass/Tile SPMD kernel).

Strategy (sharding_hint: shard nodes + edges by destination, replicate
weights, all-gather activations between layers):

  - Nodes are split into 8 contiguous blocks (padded to a multiple of 128
    rows per core).  Core c owns destination block c.
  - Per layer, using linearity of GCN aggregation:
        out_i = [sum_{j->i} dinv_i dinv_j p_j + dinv_i^2 p_i] @ W + b
    with p = previous activations.  We store ps = dinv * p ("scaled"
    activations) so every message (including the self loop, added as an
    explicit edge) has unit coefficient:
        z_i = dinv_i * segment_sum(ps[src])       (over edges + self edges)
        out_i = z_i @ W + b ; next ps = dinv * relu(out)
  - Each core DMA-gathers ps[src] rows (512B each) for its edges from a
    replicated full-activation DRAM buffer, reduces them into per-256-dst
    "window pair" PSUM tiles with one-hot matmuls (one-hots built by DVE
    is_equal against an iota row), applies dinv, multiplies by W (PE),
    bias+ReLU (ACT), rescales, and writes its 1/8 output block.
  - An AllGather (ncfw collective) replicates the per-core ps blocks
    between layers.  Matmul operands use float32r (TF32-like, ~1e-4 rel).

Edges are sorted by (dst core, dst window pair, src chunk); each
(pair, chunk) cell is padded to whole 128-token groups with a group count
equalized across cores so a single SPMD program serves all 8 cores.
Gather indices are int16 (chunk-relative, chunks of <=32768 rows).
"""

import sys

if "/opt/trn_rl_repo" not in sys.path:
    sys.path.insert(0, "/opt/trn_rl_repo")

import numpy as np

import concourse.bacc as bacc
import concourse.mybir as mybir
import concourse.tile as tile
from concourse import bass_utils

F32 = mybir.dt.float32
F32R = mybir.dt.float32r
F16 = mybir.dt.float16
I16 = mybir.dt.int16

NCORES = 8
D = 128
USE_F32R = True      # float32r (TF32-like) matmul operands; False = exact fp32
USE_F16_MSG = True   # fp16 gathered messages + one-hot S (halves gather bytes)
NQ = 4               # SWDGE queues for gather desc-gen parallelism
PAIRW = 256          # dst window-pair width (S matrix / PSUM free size)
CHUNK_ROWS = 25088   # gather source chunk rows (<= 32768 for int16 idx)
RP = 2               # window pairs per gather range


def _preprocess(edge_index, n_nodes):
    """Host-side integer preprocessing: degrees, edge partition, padding.

    Returns a dict with the static program structure (identical across
    cores) and per-core gather/one-hot metadata arrays.
    """
    src = np.asarray(edge_index[0], dtype=np.int64)
    dst = np.asarray(edge_index[1], dtype=np.int64)

    nb_real = -(-n_nodes // NCORES)              # real rows per core
    NB = -(-nb_real // 128) * 128                # padded rows per core
    NPAD = NB * NCORES
    NW = NB // 128                               # windows per core
    NPAIR = -(-NW // 2)                          # window pairs per core
    nchunks = -(-NPAD // CHUNK_ROWS)

    deg = np.ones(n_nodes, dtype=np.float64)
    np.add.at(deg, dst, 1.0)                     # bincount, +1 self loop
    deg = deg.astype(np.float32)

    # global padded row id of each node
    def gp(n):
        return (n // nb_real) * NB + (n % nb_real)

    # append self edges
    allnodes = np.arange(n_nodes, dtype=np.int64)
    s_all = np.concatenate([src, allnodes])
    d_all = np.concatenate([dst, allnodes])

    core = d_all // nb_real
    dloc = d_all % nb_real
    pair = dloc // PAIRW
    poff = dloc - pair * PAIRW                   # offset within pair [0, 256)
    sgp = gp(s_all)
    chunk = sgp // CHUNK_ROWS
    sidx = (sgp - chunk * CHUNK_ROWS).astype(np.int64)

    # sort by (core, pair, chunk), then src within each cell (HBM locality)
    key = ((core * NPAIR) + pair) * nchunks + chunk
    order = np.lexsort((sidx, key))
    key_s = key[order]
    sidx_s = sidx[order]
    poff_s = poff[order]

    ncells = NPAIR * nchunks
    # per-core per-cell counts
    counts = np.zeros((NCORES, ncells), dtype=np.int64)
    uk, uc = np.unique(key_s, return_counts=True)
    counts.reshape(-1)[uk] = uc
    gcell = (-(-counts // 128)).max(axis=0)      # equalized group counts
    gcell = gcell.reshape(NPAIR, nchunks)        # [pair, chunk]

    gtot = int(gcell.sum())
    # stream layout: for p in pairs: for k in chunks: gcell[p,k] groups
    cell_goff = np.zeros((NPAIR, nchunks), dtype=np.int64)
    g = 0
    for p in range(NPAIR):
        for k in range(nchunks):
            cell_goff[p, k] = g
            g += gcell[p, k]

    # per-core padded token arrays in stream order (token-major flat)
    dstw_flat = np.full((NCORES, gtot * 128), -1.0, dtype=np.float32)

    # chunk stream group offsets (within each chunk's gather stream)
    chunk_goff = np.zeros((NPAIR, nchunks), dtype=np.int64)
    acc = np.zeros(nchunks, dtype=np.int64)
    for p in range(NPAIR):
        for k in range(nchunks):
            chunk_goff[p, k] = acc[k]
            acc[k] += gcell[p, k]
    gchunk = acc                                  # groups per chunk stream

    idx_streams = [
        np.zeros((NCORES, int(gchunk[k]) * 128), dtype=np.int16)
        for k in range(nchunks)
    ]

    cell_starts = np.zeros(NCORES * ncells + 1, dtype=np.int64)
    np.cumsum(counts.reshape(-1), out=cell_starts[1:])
    for c in range(NCORES):
        for p in range(NPAIR):
            for k in range(nchunks):
                cell = (c * NPAIR + p) * nchunks + k
                t0, t1 = cell_starts[cell], cell_starts[cell + 1]
                n = t1 - t0
                gk0 = chunk_goff[p, k] * 128
                idx_streams[k][c, gk0 : gk0 + n] = sidx_s[t0:t1]
                g0 = cell_goff[p, k]
                dstw_flat[c, g0 * 128 : g0 * 128 + n] = poff_s[t0:t1]
                # pads keep idx 0 / dstw -1

    # dstw: token t of group g -> [t%128, g]
    dstw = np.ascontiguousarray(
        dstw_flat.reshape(NCORES, gtot, 128).transpose(0, 2, 1)
    )

    # wrap indices: token i -> [i%16, i//16], replicated to 128 partitions
    idx_wrapped = []
    for k in range(nchunks):
        st = idx_streams[k]
        cols = st.shape[1] // 16
        w = st.reshape(NCORES, cols, 16).transpose(0, 2, 1)  # [C,16,cols]
        idx_wrapped.append(np.tile(w, (1, 8, 1)).copy())     # [C,128,cols]

    # degree layouts
    deg_pad = np.ones(NPAD, dtype=np.float32)
    for c in range(NCORES):
        lo = c * nb_real
        hi = min(n_nodes, (c + 1) * nb_real)
        deg_pad[c * NB : c * NB + (hi - lo)] = deg[lo:hi]
    deg_w = np.empty((NCORES, 128, NW), dtype=np.float32)    # wrapped
    deg_r = np.empty((NCORES, 1, NB), dtype=np.float32)      # row
    for c in range(NCORES):
        blk = deg_pad[c * NB : (c + 1) * NB]
        deg_w[c] = blk.reshape(NW, 128).T
        deg_r[c, 0] = blk

    return dict(
        NB=NB, NPAD=NPAD, NW=NW, NPAIR=NPAIR, nchunks=nchunks,
        nb_real=nb_real, gcell=gcell, gtot=gtot, gchunk=gchunk,
        cell_goff=cell_goff, chunk_goff=chunk_goff,
        idx_wrapped=idx_wrapped, dstw=dstw, deg_w=deg_w, deg_r=deg_r,
    )


def _build(meta):
    NB, NPAD, NW, NPAIR = meta["NB"], meta["NPAD"], meta["NW"], meta["NPAIR"]
    nchunks, gcell, gtot = meta["nchunks"], meta["gcell"], meta["gtot"]
    gchunk, cell_goff, chunk_goff = (
        meta["gchunk"], meta["cell_goff"], meta["chunk_goff"],
    )

    DT_R = F32R if USE_F32R else F32
    DT_M = F16 if USE_F16_MSG else DT_R     # message/S dtype

    nc = bacc.Bacc(None, target_bir_lowering=False, num_devices=NCORES,
                   num_swdge_queues=NQ)

    x_ext = nc.dram_tensor("x", [NB, D], F32, kind="ExternalInput")
    degw_ext = nc.dram_tensor("degw", [128, NW], F32, kind="ExternalInput")
    iota_ext = nc.dram_tensor("iota", [128, PAIRW],
                              F16 if USE_F16_MSG else F32,
                              kind="ExternalInput")
    ident_ext = nc.dram_tensor("ident", [128, 128], F32, kind="ExternalInput")
    ones_ext = nc.dram_tensor("ones1", [1, 128], F32, kind="ExternalInput")
    w_ext = [
        nc.dram_tensor(f"w{l}", [D, D], F32, kind="ExternalInput")
        for l in range(3)
    ]
    b_ext = [
        nc.dram_tensor(f"b{l}", [128, 1], F32, kind="ExternalInput")
        for l in range(3)
    ]
    idx_ext = [
        nc.dram_tensor(f"idx{k}", [128, int(gchunk[k]) * 8], I16,
                       kind="ExternalInput")
        for k in range(nchunks)
    ]
    dstw_ext = nc.dram_tensor("dstw", [128, gtot], F32, kind="ExternalInput")
    out_ext = nc.dram_tensor("out", [NB, D], F32, kind="ExternalOutput")

    ps_loc = nc.dram_tensor("ps_loc", [NB, D], DT_M)
    ps_full = nc.dram_tensor("ps_full", [NPAD, D], DT_M, addr_space="Shared")

    QROT = [0]

    # gather ranges: RP pairs each
    ranges = [list(range(r, min(r + RP, NPAIR))) for r in range(0, NPAIR, RP)]

    with tile.TileContext(nc) as tc:
        with (
            tc.tile_pool(name="const", bufs=1) as cpool,
            tc.tile_pool(name="msg", bufs=3) as mpool,
            tc.tile_pool(name="idxp", bufs=2) as ipool,
            tc.tile_pool(name="sbld", bufs=8) as spool,
            tc.tile_pool(name="work", bufs=3) as wpool,
            tc.tile_pool(name="outp", bufs=4) as opool,
            tc.tile_pool(name="pz", bufs=2, space="PSUM") as pzpool,
            tc.tile_pool(name="pt", bufs=2, space="PSUM") as ptpool,
            tc.tile_pool(name="ph", bufs=2, space="PSUM") as phpool,
        ):
            # ---- constants ----
            iota_sb = cpool.tile([128, PAIRW], F16 if USE_F16_MSG else F32)
            nc.sync.dma_start(out=iota_sb[:], in_=iota_ext[:, :])
            ident_sb = cpool.tile([128, 128], F32)
            nc.sync.dma_start(out=ident_sb[:], in_=ident_ext[:, :])
            ones_sb = cpool.tile([1, 128], F32)
            nc.sync.dma_start(out=ones_sb[:], in_=ones_ext[:, :])
            w_sb = []
            for l in range(3):
                wt = cpool.tile([D, D], F32, tag=f"wraw{l}")
                nc.sync.dma_start(out=wt[:], in_=w_ext[l][:, :])
                if USE_F32R:
                    wr = cpool.tile([D, D], F32R, tag=f"w{l}")
                    nc.vector.tensor_copy(wr[:], wt[:])
                    w_sb.append(wr)
                else:
                    w_sb.append(wt)
            b_sb = []
            for l in range(3):
                bt = cpool.tile([128, 1], F32, tag=f"b{l}")
                nc.sync.dma_start(out=bt[:], in_=b_ext[l][:, :])
                b_sb.append(bt)
            dstw_sb = cpool.tile([128, gtot], F32)
            nc.sync.dma_start(out=dstw_sb[:], in_=dstw_ext[:, :])

            # ---- dinv (wrapped + broadcast along free dim) ----
            degw_sb = cpool.tile([128, NW], F32)
            nc.sync.dma_start(out=degw_sb[:], in_=degw_ext[:, :])
            rcpw = cpool.tile([128, NW], F32)
            nc.vector.reciprocal(rcpw[:], degw_sb[:])
            dinv_w = cpool.tile([128, NW], F32)
            nc.scalar.activation(dinv_w[:], rcpw[:],
                                 mybir.ActivationFunctionType.Sqrt)

            # dinv_bc[p, d] = dinv[d]: per window, move the dinv column to a
            # partition-0 row (matmul vs identity), then K=1 ones-broadcast.
            dinv_bc = cpool.tile([128, NB], F32)
            for w in range(NW):
                pr = ptpool.tile([128, 128], F32, tag="tp")
                nc.tensor.matmul(pr[0:1, :], dinv_w[:, w : w + 1],
                                 ident_sb[:], start=True, stop=True)
                row_sb = wpool.tile([1, 128], F32, tag="drow")
                nc.scalar.copy(out=row_sb[:], in_=pr[0:1, :])
                pb = ptpool.tile([128, 128], F32, tag="tp")
                nc.tensor.matmul(pb[:], ones_sb[:], row_sb[:],
                                 start=True, stop=True)
                nc.scalar.copy(
                    out=dinv_bc[:, w * 128 : w * 128 + 128], in_=pb[:]
                )

            # ---- prologue: ps0 = dinv * x ----
            for w in range(NW):
                xt = wpool.tile([128, 128], F32, tag="xin")
                nc.sync.dma_start(out=xt[:], in_=x_ext[w * 128 : w * 128 + 128, :])
                xs = opool.tile([128, 128], DT_M, tag="psout")
                nc.vector.tensor_scalar(
                    xs[:], xt[:], dinv_w[:, w : w + 1], None,
                    op0=mybir.AluOpType.mult,
                )
                nc.sync.dma_start(
                    out=ps_loc[w * 128 : w * 128 + 128, :], in_=xs[:]
                )
            nc.gpsimd.collective_compute(
                "AllGather", mybir.AluOpType.bypass,
                replica_groups=[list(range(NCORES))],
                ins=[ps_loc.ap().opt()], outs=[ps_full.ap().opt()],
            )

            # ---- layers ----
            for layer in range(3):
                for rng_pairs in ranges:
                    # gather all chunks for this range
                    mtiles = {}
                    for k in range(nchunks):
                        g_rk = int(sum(gcell[p, k] for p in rng_pairs))
                        if g_rk == 0:
                            continue
                        g0 = int(chunk_goff[rng_pairs[0], k])
                        ni = g_rk * 128
                        it = ipool.tile([128, ni // 16], I16, tag=f"i{k}")
                        nc.sync.dma_start(
                            out=it[:],
                            in_=idx_ext[k][:, g0 * 8 : g0 * 8 + ni // 16],
                        )
                        mt = mpool.tile([128, g_rk, 128], DT_M, tag=f"m{k}")
                        nc.gpsimd.dma_gather(
                            mt[:],
                            ps_full[k * CHUNK_ROWS : (k + 1) * CHUNK_ROWS, :],
                            it[:],
                            ni, ni, D,
                            single_packet=False,
                            queue_num=QROT[0] % NQ,
                        )
                        QROT[0] += 1
                        mtiles[k] = (mt, g0)

                    for p in rng_pairs:
                        # segment-sum into PSUM [feat, PAIRW]
                        zps = pzpool.tile([128, PAIRW], F32, tag="zacc")
                        ng = int(sum(gcell[p, k] for k in range(nchunks)))
                        gi = 0
                        for k in range(nchunks):
                            for j in range(int(gcell[p, k])):
                                mt, g0 = mtiles[k]
                                slot = int(chunk_goff[p, k]) - g0 + j
                                gcol = int(cell_goff[p, k]) + j
                                s_t = spool.tile([128, PAIRW], DT_M, tag="s")
                                nc.vector.tensor_scalar(
                                    s_t[:], iota_sb[:],
                                    dstw_sb[:, gcol : gcol + 1], None,
                                    op0=mybir.AluOpType.is_equal,
                                )
                                nc.tensor.matmul(
                                    zps[:], mt[:, slot, :], s_t[:],
                                    start=(gi == 0), stop=(gi == ng - 1),
                                )
                                gi += 1

                        # z^T = dinv ⊙ u^T ; -> SBUF f32r (rhs of W matmul)
                        zsT = wpool.tile([128, PAIRW], DT_R, tag="zst")
                        c0 = p * PAIRW
                        nc.vector.tensor_mul(
                            zsT[:], zps[:], dinv_bc[:, c0 : c0 + PAIRW]
                        )

                        hps = phpool.tile([128, PAIRW], F32, tag="h")
                        nc.tensor.matmul(
                            hps[:], w_sb[layer][:], zsT[:],
                            start=True, stop=True,
                        )
                        hT = wpool.tile([128, PAIRW], F32, tag="ht")
                        if layer < 2:
                            nc.scalar.activation(
                                hT[:], hps[:],
                                mybir.ActivationFunctionType.Relu,
                                bias=b_sb[layer][:],
                            )
                        else:
                            nc.scalar.activation(
                                hT[:], hps[:],
                                mybir.ActivationFunctionType.Identity,
                                bias=b_sb[layer][:],
                            )
                        for h in range(2):
                            w = p * 2 + h
                            if w >= NW:
                                break
                            tp = ptpool.tile([128, 128], F32, tag="tp")
                            nc.tensor.transpose(
                                tp[:], hT[:, h * 128 : h * 128 + 128],
                                ident_sb[:],
                            )
                            if layer < 2:
                                pst = opool.tile([128, 128], DT_M, tag="psout")
                                nc.vector.tensor_scalar(
                                    pst[:], tp[:], dinv_w[:, w : w + 1], None,
                                    op0=mybir.AluOpType.mult,
                                )
                                nc.sync.dma_start(
                                    out=ps_loc[w * 128 : w * 128 + 128, :],
                                    in_=pst[:],
                                )
                            else:
                                ot = opool.tile([128, 128], F32, tag="oout")
                                nc.scalar.copy(out=ot[:], in_=tp[:])
                                nc.sync.dma_start(
                                    out=out_ext[w * 128 : w * 128 + 128, :],
                                    in_=ot[:],
                                )
                if layer < 2:
                    nc.gpsimd.collective_compute(
                        "AllGather", mybir.AluOpType.bypass,
                        replica_groups=[list(range(NCORES))],
                        ins=[ps_loc.ap().opt()], outs=[ps_full.ap().opt()],
                    )

    nc.finalize()
    return nc


_CACHE = {}
TRACE = False          # set by test harness to profile + fill LAST_EXEC_NS
LAST_EXEC_NS = None


def kernel(x, edge_index, W1, b1, W2, b2, W3, b3):
    global LAST_EXEC_NS
    x = np.asarray(x, dtype=np.float32)
    edge_index = np.asarray(edge_index)
    n_nodes = x.shape[0]

    ck = (n_nodes, edge_index.shape[1],
          hash(edge_index.tobytes()))
    if ck in _CACHE:
        meta, nc = _CACHE[ck]
    else:
        meta = _preprocess(edge_index, n_nodes)
        nc = _build(meta)
        _CACHE[ck] = (meta, nc)

    NB, NW, nb_real = meta["NB"], meta["NW"], meta["nb_real"]
    nchunks = meta["nchunks"]

    iota_dt = np.float16 if USE_F16_MSG else np.float32
    iota = np.tile(np.arange(PAIRW, dtype=iota_dt), (128, 1))
    ident = np.eye(128, dtype=np.float32)
    ones1 = np.ones((1, 128), dtype=np.float32)
    ws = [np.asarray(W1, np.float32), np.asarray(W2, np.float32),
          np.asarray(W3, np.float32)]
    bs = [np.asarray(b1, np.float32), np.asarray(b2, np.float32),
          np.asarray(b3, np.float32)]

    in_maps = []
    for c in range(NCORES):
        lo = c * nb_real
        hi = min(n_nodes, (c + 1) * nb_real)
        xb = np.zeros((NB, D), dtype=np.float32)
        xb[: hi - lo] = x[lo:hi]
        im = {
            "x": xb,
            "degw": meta["deg_w"][c],
            "iota": iota,
            "ident": ident,
            "ones1": ones1,
            "dstw": meta["dstw"][c],
        }
        for l in range(3):
            im[f"w{l}"] = ws[l]
            im[f"b{l}"] = bs[l].reshape(128, 1)
        for k in range(nchunks):
            im[f"idx{k}"] = meta["idx_wrapped"][k][c]
        in_maps.append(im)

    res = bass_utils.run_bass_kernel_spmd(
        nc, in_maps, core_ids=list(range(NCORES)), trace=TRACE
    )
    LAST_EXEC_NS = res.exec_time_ns

    out = np.empty((n_nodes, D), dtype=np.float32)
    for c in range(NCORES):
        lo = c * nb_real
        hi = min(n_nodes, (c + 1) * nb_real)
        out[lo:hi] = res.results[c]["out"][: hi - lo]
    return out



# revision 2
# speedup vs baseline: 1.0005x; 1.0005x over previous
"""3-layer GCN on 8 Trainium2 NeuronCores — v2 (transform-first, PAIRW=128).

Design vs baseline:
  - Apply W BEFORE aggregation (linearity): per layer, each core transforms
    its own activation block (m_all = q @ W with q = dinv*relu(prev)), so the
    segment-sum output IS the layer output. No post-aggregation W matmul, no
    transposes in the hot loop.
  - Seg-sum matmuls use the one-hot S as the STATIONARY operand with
    128-wide destination windows: 128 cycles per 128-edge group (1 cyc/edge,
    2x the baseline), output lands directly in [dst, feat] layout.
  - One-hots for a whole window are built with ONE batched DVE tensor_tensor
    is_equal against a broadcast dstw (vs per-group tensor_scalar).
  - Self-loop tokens removed from the edge stream (folded into the epilogue
    via the SBUF-resident m_all), bias folded into the epilogue STT op.
  - AllGather split into 4 quarter collectives, double-buffered ps_full, so
    collectives overlap the previous layer's tail compute.
"""

import sys

if "/opt/trn_rl_repo" not in sys.path:
    sys.path.insert(0, "/opt/trn_rl_repo")

import numpy as np

import concourse.bacc as bacc
from concourse.bass import InstructionNameOrderedSet
import concourse.mybir as mybir
import concourse.tile as tile
from concourse import bass_utils

F32 = mybir.dt.float32
F16 = mybir.dt.float16
I16 = mybir.dt.int16

NCORES = 8
D = 128
NBR = 12500        # real rows per core
QREAL = 3125       # real rows per quarter
QR = 3200          # padded rows per quarter
NB = 12800         # padded rows per core
NW = 100           # windows per core
WQ = 25            # windows per quarter
CHUNK = 25600      # rows per chunk table (8 cores x QR)
NCH = 4
RW = 5             # windows per gather range
NR = NW // RW      # ranges
NQ = 4             # swdge queues (gathers chained in-order so the DMASW sem-lane round-robin stays queue-aligned)
SP = False         # single_packet for gathers


def _preprocess(edge_index):
    src = np.asarray(edge_index[0], dtype=np.int64)
    dst = np.asarray(edge_index[1], dtype=np.int64)
    ne = src.shape[0]

    deg = np.ones(100000, dtype=np.float64)
    np.add.at(deg, dst, 1.0)
    dinv = (1.0 / np.sqrt(deg)).astype(np.float32)

    # node -> (core, chunk, chunk-row, local padded row)
    c = src // NBR
    r = src % NBR
    q = r // QREAL
    off = r - q * QREAL
    s_chunk = q.astype(np.int64)
    s_idx = (c * QR + off).astype(np.int64)

    cd = dst // NBR
    rd = dst % NBR
    qd = rd // QREAL
    offd = rd - qd * QREAL
    lrow = qd * QR + offd
    d_win = lrow // 128
    d_off = lrow - d_win * 128

    # sort per (core, window, chunk, src)
    key = ((cd * NW + d_win) * NCH + s_chunk)
    order = np.lexsort((s_idx, key))
    key_s = key[order]
    sidx_s = s_idx[order]
    doff_s = d_off[order]

    ncell = NW * NCH
    counts = np.zeros(NCORES * ncell, dtype=np.int64)
    uk, uc = np.unique(key_s, return_counts=True)
    counts[uk] = uc
    counts = counts.reshape(NCORES, NW, NCH)
    gcell = -(-counts.max(axis=0) // 128)          # [NW, NCH] equalized
    assert (gcell > 0).all()

    # group bookkeeping
    ng_w = gcell.sum(axis=1)                        # groups per window
    NGMAX = int(ng_w.max())
    g0_w = np.zeros(NW + 1, dtype=np.int64)
    np.cumsum(ng_w, out=g0_w[1:])
    gtot = int(g0_w[-1])

    # grk[R, k] = groups of chunk k in range R ; slot offsets per (w, k)
    grk = np.zeros((NR, NCH), dtype=np.int64)
    slot_wk = np.zeros((NW, NCH), dtype=np.int64)   # slot base within (R,k)
    for R in range(NR):
        for k in range(NCH):
            acc = 0
            for w in range(R * RW, (R + 1) * RW):
                slot_wk[w, k] = acc
                acc += gcell[w, k]
            grk[R, k] = acc
    GRK = int(grk.max())

    # chunk idx stream offsets: chunk k laid out in (R, w, j) order
    koff = np.zeros((NR, NCH), dtype=np.int64)      # group offset of (R,k)
    acc = np.zeros(NCH, dtype=np.int64)
    for R in range(NR):
        for k in range(NCH):
            koff[R, k] = acc[k]
            acc[k] += grk[R, k]
    glen = acc                                       # groups per chunk stream

    # fill idx streams + dstw
    # pad tokens gather the dead (zero) row 3125 of core 0's quarter;
    # their dstw = -1 so the one-hot row is all-zero regardless.
    idx_streams = [np.full((NCORES, int(glen[k]) * 128), QREAL,
                           dtype=np.int16) for k in range(NCH)]
    dstw_flat = np.full((NCORES, gtot * 128), -1.0, dtype=np.float16)

    cell_starts = np.zeros(NCORES * ncell + 1, dtype=np.int64)
    np.cumsum(counts.reshape(-1), out=cell_starts[1:])
    for core in range(NCORES):
        for w in range(NW):
            R = w // RW
            gbase = int(g0_w[w])
            goff_in_w = 0
            for k in range(NCH):
                cell = (core * NW + w) * NCH + k
                t0, t1 = cell_starts[cell], cell_starts[cell + 1]
                n = int(t1 - t0)
                sg0 = (koff[R, k] + slot_wk[w, k]) * 128
                idx_streams[k][core, sg0: sg0 + n] = sidx_s[t0:t1]
                dg0 = (gbase + goff_in_w) * 128
                dstw_flat[core, dg0: dg0 + n] = doff_s[t0:t1]
                goff_in_w += int(gcell[w, k])

    # wrap: dstw [C, 128, gtot] (partition = token-in-group)
    dstw = np.ascontiguousarray(
        dstw_flat.reshape(NCORES, gtot, 128).transpose(0, 2, 1))

    # idx wrapped [C, 128, glen_k*8]
    idx_wrapped = []
    for k in range(NCH):
        st = idx_streams[k]
        cols = st.shape[1] // 16
        wv = st.reshape(NCORES, cols, 16).transpose(0, 2, 1)
        idx_wrapped.append(np.tile(wv, (1, 8, 1)).copy())

    # dinv per core: wrapped [128, NW] (partition = slot), dead rows -> 0
    dinv_w = np.zeros((NCORES, 128, NW), dtype=np.float32)
    for core in range(NCORES):
        blk = np.zeros(NB, dtype=np.float32)
        for qq in range(4):
            lo = core * NBR + qq * QREAL
            blk[qq * QR: qq * QR + QREAL] = dinv[lo: lo + QREAL]
        dinv_w[core] = blk.reshape(NW, 128).T

    return dict(gcell=gcell, ng_w=ng_w, NGMAX=NGMAX, g0_w=g0_w, gtot=gtot,
                grk=grk, GRK=GRK, slot_wk=slot_wk, koff=koff, glen=glen,
                idx_wrapped=idx_wrapped, dstw=dstw, dinv_w=dinv_w)


def _build(meta):
    NGMAX, GRK, gtot = meta["NGMAX"], meta["GRK"], meta["gtot"]
    gcell, g0_w, grk = meta["gcell"], meta["g0_w"], meta["grk"]
    slot_wk, koff, glen = meta["slot_wk"], meta["koff"], meta["glen"]

    nc = bacc.Bacc(None, target_bir_lowering=False, num_devices=NCORES,
                   num_swdge_queues=NQ)

    x_ext = nc.dram_tensor("x", [NB, D], F32, kind="ExternalInput")
    dinvw_ext = nc.dram_tensor("dinvw", [128, NW], F32, kind="ExternalInput")
    iog_ext = nc.dram_tensor("iog", [128, NGMAX, 128], F16,
                             kind="ExternalInput")
    ident_ext = nc.dram_tensor("ident", [128, 128], F16, kind="ExternalInput")
    dstw_ext = nc.dram_tensor("dstw", [128, gtot], F16, kind="ExternalInput")
    w_ext = [nc.dram_tensor(f"w{l}", [D, D], F16, kind="ExternalInput")
             for l in range(3)]
    bbc_ext = [nc.dram_tensor(f"bbc{l}", [128, D], F32, kind="ExternalInput")
               for l in range(3)]
    idx_ext = [nc.dram_tensor(f"idx{k}", [128, int(glen[k]) * 8], I16,
                              kind="ExternalInput") for k in range(NCH)]
    out_ext = nc.dram_tensor("out", [NB, D], F32, kind="ExternalOutput")

    ps_loc = [nc.dram_tensor(f"ps_loc{q}", [QR, D], F16) for q in range(4)]
    ps_full = [[nc.dram_tensor(f"ps_full{p}_{k}", [CHUNK, D], F16,
                               addr_space="Shared")
                for k in range(NCH)] for p in range(2)]

    QROT = [0]
    PREV_G = [None]

    with tile.TileContext(nc) as tc:
        with (
            tc.tile_pool(name="const", bufs=1) as cpool,
            tc.tile_pool(name="sb", bufs=3) as spool,
            tc.tile_pool(name="msg", bufs=3) as mpool,
            tc.tile_pool(name="wk", bufs=3) as wpool,
            tc.tile_pool(name="pz", bufs=3, space="PSUM") as pzpool,
            tc.tile_pool(name="pt", bufs=2, space="PSUM") as ptpool,
            tc.tile_pool(name="pm", bufs=2, space="PSUM") as pmpool,
        ):
            # ---- constants ----
            iog = cpool.tile([128, NGMAX, 128], F16)
            nc.sync.dma_start(out=iog[:], in_=iog_ext[:, :, :])
            ident = cpool.tile([128, 128], F16)
            nc.sync.dma_start(out=ident[:], in_=ident_ext[:, :])
            dstw = cpool.tile([128, gtot], F16)
            nc.sync.dma_start(out=dstw[:], in_=dstw_ext[:, :])
            dinvw = cpool.tile([128, NW], F32)
            nc.sync.dma_start(out=dinvw[:], in_=dinvw_ext[:, :])
            wmat = []
            for l in range(3):
                wt = cpool.tile([D, D], F16, tag=f"w{l}")
                nc.sync.dma_start(out=wt[:], in_=w_ext[l][:, :])
                wmat.append(wt)
            bbc = []
            for l in range(3):
                bt = cpool.tile([128, D], F32, tag=f"bbc{l}")
                nc.sync.dma_start(out=bt[:], in_=bbc_ext[l][:, :])
                bbc.append(bt)
            idx_sb = []
            for k in range(NCH):
                it = cpool.tile([128, int(glen[k]) * 8], I16, tag=f"idx{k}")
                nc.sync.dma_start(out=it[:], in_=idx_ext[k][:, :])
                idx_sb.append(it)
            q_t = [cpool.tile([128, 128], F16, tag=f"q{w}",
                              name=f"q{w}") for w in range(NW)]
            mall_t = [cpool.tile([128, 128], F16, tag=f"ma{w}",
                                 name=f"ma{w}") for w in range(NW)]

            # memset message tiles once (avoid NaN garbage under pad slots)
            for k in range(NCH):
                for b in range(2):
                    mt = mpool.tile([128, GRK, 128], F16, tag=f"m{k}")
                    nc.vector.memset(mt[:], 0.0)

            def prologue(w, l):
                """q[:, w] -> mall[:, w] = (q @ W_l); write ps_loc."""
                tp = ptpool.tile([128, 128], F16, tag="tp")
                nc.tensor.transpose(tp[:], q_t[w][:], ident[:])
                qt = wpool.tile([128, 128], F16, tag="qt")
                nc.scalar.copy(out=qt[:], in_=tp[:])
                mp = pmpool.tile([128, 128], F32, tag="mp")
                nc.tensor.matmul(mp[:], qt[:], wmat[l][:], start=True,
                                 stop=True)
                nc.vector.tensor_copy(mall_t[w][:], mp[:])
                qq, wq = w // WQ, w % WQ
                nc.sync.dma_start(
                    out=ps_loc[qq][wq * 128: wq * 128 + 128, :],
                    in_=mall_t[w][:])

            def allgather(qq, parity):
                nc.gpsimd.collective_compute(
                    "AllGather", mybir.AluOpType.bypass,
                    replica_groups=[list(range(NCORES))],
                    ins=[ps_loc[qq].ap().opt()],
                    outs=[ps_full[parity][qq].ap().opt()],
                )

            # ---- layer-0 pre-pass: q0 = dinv * x ; m_all0 ; AGs ----
            for w in range(NW):
                xt = wpool.tile([128, 128], F32, tag="xt")
                nc.sync.dma_start(out=xt[:],
                                  in_=x_ext[w * 128: w * 128 + 128, :])
                nc.vector.tensor_scalar(
                    q_t[w][:], xt[:], dinvw[:, w: w + 1], None,
                    op0=mybir.AluOpType.mult)
                prologue(w, 0)
                if w % WQ == WQ - 1:
                    allgather(w // WQ, 0)

            # ---- layers ----
            for l in range(3):
                par = l % 2
                for R in range(NR):
                    mts = []
                    for k in range(NCH):
                        ni = int(grk[R, k]) * 128
                        g0 = int(koff[R, k])
                        mt = mpool.tile([128, GRK, 128], F16, tag=f"m{k}")
                        g = nc.gpsimd.dma_gather(
                            mt[:, 0: int(grk[R, k]), :],
                            ps_full[par][k][0:CHUNK, :],
                            idx_sb[k][:, g0 * 8: g0 * 8 + ni // 16],
                            ni, ni, D,
                            single_packet=SP,
                            queue_num=QROT[0] % NQ,
                        )
                        if PREV_G[0] is not None:
                            s = InstructionNameOrderedSet()
                            s.add(PREV_G[0])
                            g.ins.add_nosync_dependencies_from(s)
                        PREV_G[0] = g.ins.name
                        QROT[0] += 1
                        mts.append(mt)
                    for w in range(R * RW, (R + 1) * RW):
                        ngw = int(g0_w[w + 1] - g0_w[w])
                        st = spool.tile([128, NGMAX, 128], F16, tag="s")
                        nc.vector.tensor_tensor(
                            st[:, 0:ngw, :],
                            iog[:, 0:ngw, :],
                            dstw[:, int(g0_w[w]): int(g0_w[w]) + ngw, None]
                            .to_broadcast([128, ngw, 128]),
                            mybir.AluOpType.is_equal)
                        zp = pzpool.tile([128, 128], F32, tag="z")
                        gi = 0
                        col = 0
                        for k in range(NCH):
                            for j in range(int(gcell[w, k])):
                                slot = int(slot_wk[w, k]) + j
                                nc.tensor.matmul(
                                    zp[:], st[:, col, :],
                                    mts[k][:, slot, :],
                                    start=(gi == 0), stop=(gi == ngw - 1))
                                gi += 1
                                col += 1
                        # epilogue
                        y1 = wpool.tile([128, 128], F32, tag="y1")
                        nc.vector.tensor_tensor(
                            y1[:], zp[:], mall_t[w][:],
                            mybir.AluOpType.add)
                        y2 = wpool.tile([128, 128], F32, tag="y2")
                        nc.vector.scalar_tensor_tensor(
                            y2[:], y1[:], dinvw[:, w: w + 1], bbc[l][:],
                            op0=mybir.AluOpType.mult,
                            op1=mybir.AluOpType.add)
                        if l < 2:
                            nc.scalar.activation(
                                q_t[w][:], y2[:],
                                mybir.ActivationFunctionType.Relu,
                                scale=dinvw[:, w: w + 1])
                            prologue(w, l + 1)
                            if w % WQ == WQ - 1:
                                allgather(w // WQ, 1 - par)
                        else:
                            ot = wpool.tile([128, 128], F32, tag="ot")
                            nc.vector.tensor_copy(ot[:], y2[:])
                            nc.sync.dma_start(
                                out=out_ext[w * 128: w * 128 + 128, :],
                                in_=ot[:])

    nc.finalize()
    return nc


_CACHE = {}
TRACE = False
LAST_EXEC_NS = None


def _prep_inputs(meta, x, Ws, bs):
    iog = np.broadcast_to(
        np.arange(128, dtype=np.float16),
        (128, meta["NGMAX"], 128)).copy()
    ident = np.eye(128, dtype=np.float16)
    ws = [np.asarray(w, np.float16) for w in Ws]
    bs = [np.asarray(b, np.float32) for b in bs]

    in_maps = []
    for c in range(NCORES):
        xb = np.zeros((NB, D), dtype=np.float32)
        for q in range(4):
            lo = c * NBR + q * QREAL
            xb[q * QR: q * QR + QREAL] = x[lo: lo + QREAL]
        im = {
            "x": xb,
            "dinvw": meta["dinv_w"][c],
            "iog": iog,
            "ident": ident,
            "dstw": meta["dstw"][c],
        }
        for l in range(3):
            im[f"w{l}"] = ws[l]
            im[f"bbc{l}"] = np.broadcast_to(bs[l], (128, D)).copy()
        for k in range(NCH):
            im[f"idx{k}"] = meta["idx_wrapped"][k][c]
        in_maps.append(im)
    return in_maps


def kernel(x, edge_index, W1, b1, W2, b2, W3, b3):
    global LAST_EXEC_NS
    x = np.asarray(x, dtype=np.float32)
    edge_index = np.asarray(edge_index)

    ck = (x.shape[0], edge_index.shape[1], hash(edge_index.tobytes()))
    if ck in _CACHE:
        meta, nc = _CACHE[ck]
    else:
        meta = _preprocess(edge_index)
        nc = _build(meta)
        _CACHE[ck] = (meta, nc)

    in_maps = _prep_inputs(meta, x, [W1, W2, W3], [b1, b2, b3])
    res = bass_utils.run_bass_kernel_spmd(
        nc, in_maps, core_ids=list(range(NCORES)), trace=TRACE
    )
    LAST_EXEC_NS = res.exec_time_ns

    out = np.empty((100000, D), dtype=np.float32)
    for c in range(NCORES):
        for q in range(4):
            lo = c * NBR + q * QREAL
            out[lo: lo + QREAL] = res.results[c]["out"][
                q * QR: q * QR + QREAL]
    return out
